# revision 16
# baseline (speedup 1.0000x reference)
"""Trainium2 Bass kernel for nn_BCIM_45861660787130 (pooling / box-filter sim).

Math per sample (C=128 channels, 32x32 spatial = S=1024 pixels):
  unit = p / ||p||_C
  wmean = 3x3 zero-padded box mean of unit (per channel)
  sim = <unit, wmean>_C          # per pixel
  out = p * sim, then channel deinterleave c=(f*2+e) -> [e*S + s, f]

v2 design (bf16 end-to-end, j-interleaved spatial chunking):
  - Host casts p to bf16 and pre-arranges [NG, 128c, NS, 1024s] per core, so
    the input DMA is one fully contiguous 2MB transfer per group.
  - Spatial index is chunked s = 8q + j (q in [0,128) partitions, j in [0,8)
    chunks).  Partition q then holds 8 consecutive output rows, so the output
    DMA per group is one contiguous 16KB-per-partition transfer.
  - PE transpose per (b, j): regular bf16 matmul  out[q, c'] = p_bj^T @ P
    where P is the channel-deinterleave permutation (c=2f+e -> c'=e*64+f),
    folding the output deinterleave into the transpose for free.
  - 3x3 box filter on PE: with j-interleaving the band matrices collapse to
    ONE matrix Wd (offsets {0,+-4} over q) for all chunk pairs except the two
    w-border wraps (j=0 <- chunk 7 via Wp, j=7 <- chunk 0 via Wn).
  - ACT: Square (ss path), sqrt, box PSUM->SBUF bf16 evac.
    DVE: ss-reduce, reciprocal, u-copy (x rinv broadcast), wscr=u*box,
    z-reduce, fs=z*nrm, out=u*fs (broadcast).  All batched over NS=8 samples.
  - out[b] = u * (z*nrm) = p * sim.
"""

import sys

sys.path.insert(0, "/opt/trn_rl_repo")

import numpy as np
import ml_dtypes

from concourse import bacc, mybir, tile
from concourse.bass_utils import run_bass_kernel_spmd

F32 = mybir.dt.float32
BF16 = mybir.dt.bfloat16
AF = mybir.ActivationFunctionType
ALU = mybir.AluOpType
AX = mybir.AxisListType

NPBF16 = ml_dtypes.bfloat16

import os

TRANS_IT = os.environ.get("TRANS_IT", "1") == "1"
BOX_IT = os.environ.get("BOX_IT", "0") == "1"

B_PER_CORE = 32
NS = 8          # samples per group
NG = B_PER_CORE // NS
NJ = 8          # j-chunks (s = 8q + j)
C = 128
S = 1024
Q = 128


def _consts():
    # channel deinterleave permutation: c = 2f+e  ->  c' = e*64 + f
    perm = np.zeros((128, 128), np.float32)
    for e in range(2):
        for f in range(64):
            perm[2 * f + e, e * 64 + f] = 1.0
    # band matrices over q (s = 8q + j; q = h*4 + w//8, so h-step = +-4 in q)
    qq = np.arange(Q)
    wd = np.zeros((Q, Q), np.float32)
    for d in (-4, 0, 4):
        idx = qq + d
        m = (idx >= 0) & (idx < Q)
        wd[idx[m], qq[m]] = 1.0
    wp = np.zeros((Q, Q), np.float32)   # output chunk 0 <- input chunk 7
    for dh in (-1, 0, 1):
        idx = qq - 1 + 4 * dh
        m = (idx >= 0) & (idx < Q) & (qq % 4 >= 1)
        wp[idx[m], qq[m]] = 1.0
    wn = np.zeros((Q, Q), np.float32)   # output chunk 7 <- input chunk 0
    for dh in (-1, 0, 1):
        idx = qq + 1 + 4 * dh
        m = (idx >= 0) & (idx < Q) & (qq % 4 <= 2)
        wn[idx[m], qq[m]] = 1.0
    wmat = (np.stack([wd, wp, wn]) / 9.0).astype(NPBF16)
    return perm.astype(NPBF16), wmat


def build_nc():
    TDT = BF16 if TRANS_IT else F32
    BDT = BF16 if BOX_IT else F32
    nc = bacc.Bacc()
    p_d = nc.declare_dram_parameter("p", [NG, C, NS, S], BF16, isOutput=False)
    out_d = nc.declare_dram_parameter(
        "out", [NG, Q, NS, 2, NJ, 64], BF16, isOutput=True
    )
    perm_d = nc.declare_dram_parameter("perm", [128, 128], BF16, isOutput=False)
    wmat_d = nc.declare_dram_parameter("wmat", [3, 128, 128], BF16, isOutput=False)

    with (
        nc.allow_low_precision(reason="bf16 pipeline; DVE accumulates in f32"),
        tile.TileContext(nc) as tc,
    ):
        with (
            tc.tile_pool(name="consts", bufs=1) as cpool,
            tc.tile_pool(name="pin", bufs=2) as pin,
            tc.tile_pool(name="sq", bufs=3) as sqpool,
            tc.tile_pool(name="upool", bufs=2 * NJ) as upool,
            tc.tile_pool(name="boxs", bufs=3) as bxpool,
            tc.tile_pool(name="wscr", bufs=3) as wpool,
            tc.tile_pool(name="outp", bufs=2) as outpool,
            tc.tile_pool(name="stats", bufs=8 * NJ) as stats,
            tc.tile_pool(name="psT", bufs=2, space="PSUM") as psT,
            tc.tile_pool(name="psB", bufs=3, space="PSUM") as psB,
        ):
            perm = cpool.tile([128, 128], BF16, tag="perm")
            wmat = cpool.tile([128, 3, 128], BF16, tag="wmat")
            nc.sync.dma_start(perm[:], perm_d[:])
            nc.sync.dma_start(wmat[:], wmat_d[:].transpose([1, 0, 2]))
            wd, wp, wn = wmat[:, 0, :], wmat[:, 1, :], wmat[:, 2, :]

            # startup observers: make PE's vector clock see both const-DMA
            # queue sems so steady-state matmuls never wait on them (matmuls
            # only support a single sync wait in codegen).
            scr1 = psT.tile([128, 1], F32, tag="pT")
            nc.tensor.matmul(scr1[:], perm[:], perm[:, 0:1], start=True, stop=True)
            scr2 = psT.tile([128, 1], F32, tag="pT")
            nc.tensor.matmul(scr2[:], perm[:], wmat[:, 0, 0:1], start=True, stop=True)

            pts = []
            for g in range(NG):
                pt = pin.tile([C, NS, S], BF16, tag="pt", name=f"pt_{g}")
                nc.sync.dma_start(pt[:, 0:4, :], p_d[g, :, 0:4, :])
                nc.sync.dma_start(pt[:, 4:8, :], p_d[g, :, 4:8, :])
                pts.append(pt)

            # Per-group state; norm phase of group g+1 is interleaved with
            # the box phase of group g so PE/ACT/DVE never drain between
            # groups.
            outts = [
                outpool.tile([Q, NS, 2, NJ, 64], BF16, tag="ot", name=f"ot_{g}")
                for g in range(NG)
            ]
            us = [dict() for _ in range(NG)]
            nrms = [dict() for _ in range(NG)]

            def norm_stage(g, j):
                pt = pts[g]
                pT = psT.tile([128, NS, 128], TDT, tag="pT")
                for b in range(NS):
                    sl = pt[:, b, :].rearrange("c (q j) -> c j q", j=NJ)[:, j, :]
                    nc.tensor.matmul(
                        pT[:, b, :], sl, perm[:], start=True, stop=True,
                        is_transpose=TRANS_IT,
                    )
                sq = sqpool.tile([128, NS, 128], BF16, tag="sq")
                nc.scalar.activation(sq[:], pT[:], AF.Square)
                sqf = sqpool.tile([128, NS, 64], BF16, tag="sqf")
                nc.vector.tensor_tensor(
                    sqf[:], sq[:, :, 0:64], sq[:, :, 64:128], op=ALU.add
                )
                ss = stats.tile([128, NS], BF16, tag="ss")
                nc.vector.tensor_reduce(ss[:], sqf[:], axis=AX.X, op=ALU.add)
                # duplicated-pair stats: [128, NS, 2] with both lanes equal, so
                # broadcast APs can expose a packed inner [stride 1, 2] dim
                # (2x_1p needs inner step +-1 on every non-scalar operand).
                nrmd = stats.tile([128, NS, 2], BF16, tag="nrm")
                nc.scalar.sqrt(nrmd[:], ss[:].broadcast_to((128, NS, 2)))
                rinvd = stats.tile([128, NS, 2], BF16, tag="rinv")
                nc.vector.reciprocal(rinvd[:], nrmd[:])
                u = upool.tile([128, NS, 128], BF16, tag="u", name=f"u_{g}_{j}")
                nc.vector.tensor_tensor(
                    u[:],
                    pT[:],
                    rinvd[:, :, None].broadcast_to((128, NS, 64, 2)),
                    op=ALU.mult,
                )
                us[g][j], nrms[g][j] = u, nrmd

            def box_stage(g, jj):
                outt = outts[g]
                if jj == 0:
                    srcs = [(wp, 7), (wd, 0), (wd, 1)]
                elif jj == NJ - 1:
                    srcs = [(wd, jj - 1), (wd, jj), (wn, 0)]
                else:
                    srcs = [(wd, jj - 1), (wd, jj), (wd, jj + 1)]
                if True:
                    box = psB.tile([128, NS, 128], BDT, tag="box")
                    for i, (w, sj) in enumerate(srcs):
                        for h in range(2):
                            nc.tensor.matmul(
                                box[:, 4 * h : 4 * h + 4, :],
                                w,
                                us[g][sj][:, 4 * h : 4 * h + 4, :],
                                start=(i == 0),
                                stop=(i == len(srcs) - 1),
                                is_transpose=BOX_IT,
                            )
                    if BOX_IT:
                        boxs = box
                    else:
                        boxs = bxpool.tile([128, NS, 128], BF16, tag="bx")
                        nc.scalar.activation(boxs[:], box[:], AF.Copy)
                    wscr = wpool.tile([128, NS, 128], BF16, tag="w")
                    nc.vector.tensor_tensor(
                        wscr[:], us[g][jj][:], boxs[:], op=ALU.mult
                    )
                    wf = wpool.tile([128, NS, 64], BF16, tag="wf")
                    nc.vector.tensor_tensor(
                        wf[:], wscr[:, :, 0:64], wscr[:, :, 64:128], op=ALU.add
                    )
                    z = stats.tile([128, NS], BF16, tag="z")
                    nc.vector.tensor_reduce(z[:], wf[:], axis=AX.X, op=ALU.add)
                    fsd = stats.tile([128, NS, 2], BF16, tag="fs")
                    nc.vector.tensor_tensor(
                        fsd[:],
                        z[:].broadcast_to((128, NS, 2)),
                        nrms[g][jj][:],
                        op=ALU.mult,
                    )
                    # pair-packed broadcast (fsd lanes equal) so 2x_1p can
                    # trigger; split by e to stay within 3 free dims per AP.
                    for e in range(2):
                        nc.vector.tensor_tensor(
                            outt[:, :, e, jj, :].rearrange(
                                "p n (fo t) -> p n fo t", t=2
                            ),
                            us[g][jj][:, :, e * 64 : (e + 1) * 64].rearrange(
                                "p n (fo t) -> p n fo t", t=2
                            ),
                            fsd[:, :, None].broadcast_to((128, NS, 32, 2)),
                            op=ALU.mult,
                        )

            # self-interleaved schedule: box(g, j-1) rides along norm(g, j),
            # with only the two wrap stages (7, 0) trailing each group; the
            # next group's norm stages queue right behind so engines never
            # drain at group boundaries.
            for g in range(NG):
                norm_stage(g, 0)
                norm_stage(g, 1)
                for j in range(2, NJ):
                    norm_stage(g, j)
                    box_stage(g, j - 1)
                box_stage(g, NJ - 1)
                box_stage(g, 0)
                nc.sync.dma_start(out_d[g], outts[g][:])

    nc.compile()
    return nc


def _run(p_vector: np.ndarray, **spmd_kwargs):
    p = np.ascontiguousarray(p_vector, dtype=np.float32)
    assert p.shape == (256, 128, 32, 32)
    # [core, NG, NS, C, S] -> [core, NG, C, NS, S], cast bf16
    shards = (
        p.reshape(8, NG, NS, C, S).transpose(0, 1, 3, 2, 4).astype(NPBF16)
    )
    shards = np.ascontiguousarray(shards)
    perm, wmat = _consts()
    nc = build_nc()
    in_maps = [
        {"p": shards[i], "perm": perm, "wmat": wmat} for i in range(8)
    ]
    return run_bass_kernel_spmd(nc, in_maps, core_ids=list(range(8)), **spmd_kwargs)


def _assemble(res) -> np.ndarray:
    outs = []
    for r in res.results:
        o = np.asarray(r["out"]).astype(np.float32)  # [NG, Q, NS, 2, NJ, 64]
        # -> [NG, NS, 2, Q, NJ, 64] -> [B_local, 2, 1024, 64] -> [B_local, 2048, 64]
        o = o.transpose(0, 2, 3, 1, 4, 5).reshape(B_PER_CORE, 2, 1024, 64)
        outs.append(o.reshape(B_PER_CORE, 2048, 64))
    return np.concatenate(outs, axis=0)


def kernel(p_vector: np.ndarray) -> np.ndarray:
    return _assemble(_run(p_vector))


if __name__ == "__main__":
    x = np.random.randn(256, 128, 32, 32).astype(np.float32)
    y = kernel(x)
    print(y.shape, y.dtype)


# revision 17
# speedup vs baseline: 1.0129x; 1.0129x over previous
"""Trainium2 Bass kernel for nn_BCIM_45861660787130 (pooling / box-filter sim).

Math per sample (C=128 channels, 32x32 spatial = S=1024 pixels):
  unit = p / ||p||_C
  wmean = 3x3 zero-padded box mean of unit (per channel)
  sim = <unit, wmean>_C          # per pixel
  out = p * sim, then channel deinterleave c=(f*2+e) -> [e*S + s, f]

v2 design (bf16 end-to-end, j-interleaved spatial chunking):
  - Host casts p to bf16 and pre-arranges [NG, 128c, NS, 1024s] per core, so
    the input DMA is one fully contiguous 2MB transfer per group.
  - Spatial index is chunked s = 8q + j (q in [0,128) partitions, j in [0,8)
    chunks).  Partition q then holds 8 consecutive output rows, so the output
    DMA per group is one contiguous 16KB-per-partition transfer.
  - PE transpose per (b, j): regular bf16 matmul  out[q, c'] = p_bj^T @ P
    where P is the channel-deinterleave permutation (c=2f+e -> c'=e*64+f),
    folding the output deinterleave into the transpose for free.
  - 3x3 box filter on PE: with j-interleaving the band matrices collapse to
    ONE matrix Wd (offsets {0,+-4} over q) for all chunk pairs except the two
    w-border wraps (j=0 <- chunk 7 via Wp, j=7 <- chunk 0 via Wn).
  - ACT: Square (ss path), sqrt, box PSUM->SBUF bf16 evac.
    DVE: ss-reduce, reciprocal, u-copy (x rinv broadcast), wscr=u*box,
    z-reduce, fs=z*nrm, out=u*fs (broadcast).  All batched over NS=8 samples.
  - out[b] = u * (z*nrm) = p * sim.
"""

import sys

sys.path.insert(0, "/opt/trn_rl_repo")

import numpy as np
import ml_dtypes

from concourse import bacc, mybir, tile
from concourse.bass_utils import run_bass_kernel_spmd

F32 = mybir.dt.float32
BF16 = mybir.dt.bfloat16
AF = mybir.ActivationFunctionType
ALU = mybir.AluOpType
AX = mybir.AxisListType

NPBF16 = ml_dtypes.bfloat16

import os

TRANS_IT = os.environ.get("TRANS_IT", "1") == "1"
BOX_IT = os.environ.get("BOX_IT", "0") == "1"

B_PER_CORE = 32
NS = 8          # samples per group
NG = B_PER_CORE // NS
NJ = 8          # j-chunks (s = 8q + j)
C = 128
S = 1024
Q = 128


def _consts():
    # channel deinterleave permutation: c = 2f+e  ->  c' = e*64 + f
    perm = np.zeros((128, 128), np.float32)
    for e in range(2):
        for f in range(64):
            perm[2 * f + e, e * 64 + f] = 1.0
    # band matrices over q (s = 8q + j; q = h*4 + w//8, so h-step = +-4 in q)
    qq = np.arange(Q)
    wd = np.zeros((Q, Q), np.float32)
    for d in (-4, 0, 4):
        idx = qq + d
        m = (idx >= 0) & (idx < Q)
        wd[idx[m], qq[m]] = 1.0
    wp = np.zeros((Q, Q), np.float32)   # output chunk 0 <- input chunk 7
    for dh in (-1, 0, 1):
        idx = qq - 1 + 4 * dh
        m = (idx >= 0) & (idx < Q) & (qq % 4 >= 1)
        wp[idx[m], qq[m]] = 1.0
    wn = np.zeros((Q, Q), np.float32)   # output chunk 7 <- input chunk 0
    for dh in (-1, 0, 1):
        idx = qq + 1 + 4 * dh
        m = (idx >= 0) & (idx < Q) & (qq % 4 <= 2)
        wn[idx[m], qq[m]] = 1.0
    wmat = (np.stack([wd, wp, wn]) / 9.0).astype(NPBF16)
    return perm.astype(NPBF16), wmat


def build_nc():
    TDT = BF16 if TRANS_IT else F32
    BDT = BF16 if BOX_IT else F32
    nc = bacc.Bacc()
    p_d = nc.declare_dram_parameter("p", [NG, C, NS, S], BF16, isOutput=False)
    out_d = nc.declare_dram_parameter(
        "out", [NG, Q, NS, 2, NJ, 64], BF16, isOutput=True
    )
    perm_d = nc.declare_dram_parameter("perm", [128, 128], BF16, isOutput=False)
    wmat_d = nc.declare_dram_parameter("wmat", [3, 128, 128], BF16, isOutput=False)

    with (
        nc.allow_low_precision(reason="bf16 pipeline; DVE accumulates in f32"),
        tile.TileContext(nc) as tc,
    ):
        with (
            tc.tile_pool(name="consts", bufs=1) as cpool,
            tc.tile_pool(name="pin", bufs=2) as pin,
            tc.tile_pool(name="sq", bufs=3) as sqpool,
            tc.tile_pool(name="upool", bufs=2 * NJ) as upool,
            tc.tile_pool(name="boxs", bufs=3) as bxpool,
            tc.tile_pool(name="wscr", bufs=3) as wpool,
            tc.tile_pool(name="outp", bufs=2) as outpool,
            tc.tile_pool(name="stats", bufs=8 * NJ) as stats,
            tc.tile_pool(name="psT", bufs=2, space="PSUM") as psT,
            tc.tile_pool(name="psB", bufs=3, space="PSUM") as psB,
        ):
            perm = cpool.tile([128, 128], BF16, tag="perm")
            wmat = cpool.tile([128, 3, 128], BF16, tag="wmat")
            nc.sync.dma_start(perm[:], perm_d[:])
            nc.sync.dma_start(wmat[:], wmat_d[:].transpose([1, 0, 2]))
            wd, wp, wn = wmat[:, 0, :], wmat[:, 1, :], wmat[:, 2, :]

            # startup observers: make PE's vector clock see both const-DMA
            # queue sems so steady-state matmuls never wait on them (matmuls
            # only support a single sync wait in codegen).
            scr1 = psT.tile([128, 1], F32, tag="pT")
            nc.tensor.matmul(scr1[:], perm[:], perm[:, 0:1], start=True, stop=True)
            scr2 = psT.tile([128, 1], F32, tag="pT")
            nc.tensor.matmul(scr2[:], perm[:], wmat[:, 0, 0:1], start=True, stop=True)

            pts = []
            for g in range(NG):
                pt = pin.tile([C, NS, S], BF16, tag="pt", name=f"pt_{g}")
                nc.sync.dma_start(pt[:, 0:4, :], p_d[g, :, 0:4, :])
                nc.sync.dma_start(pt[:, 4:8, :], p_d[g, :, 4:8, :])
                pts.append(pt)

            # Per-group state; norm phase of group g+1 is interleaved with
            # the box phase of group g so PE/ACT/DVE never drain between
            # groups.
            outts = [
                outpool.tile([Q, NS, 2, NJ, 64], BF16, tag="ot", name=f"ot_{g}")
                for g in range(NG)
            ]
            us = [dict() for _ in range(NG)]
            nrms = [dict() for _ in range(NG)]

            def norm_stage(g, j):
                pt = pts[g]
                pT = psT.tile([128, NS, 128], TDT, tag="pT")
                for b in range(NS):
                    sl = pt[:, b, :].rearrange("c (q j) -> c j q", j=NJ)[:, j, :]
                    nc.tensor.matmul(
                        pT[:, b, :], sl, perm[:], start=True, stop=True,
                        is_transpose=TRANS_IT,
                    )
                sq = sqpool.tile([128, NS, 128], BF16, tag="sq")
                nc.scalar.activation(sq[:], pT[:], AF.Square)
                sqf = sqpool.tile([128, NS, 64], BF16, tag="sqf")
                nc.vector.tensor_tensor(
                    sqf[:], sq[:, :, 0:64], sq[:, :, 64:128], op=ALU.add
                )
                ss = stats.tile([128, NS], BF16, tag="ss")
                nc.vector.tensor_reduce(ss[:], sqf[:], axis=AX.X, op=ALU.add)
                # duplicated-pair stats: [128, NS, 2] with both lanes equal, so
                # broadcast APs can expose a packed inner [stride 1, 2] dim
                # (2x_1p needs inner step +-1 on every non-scalar operand).
                nrmd = stats.tile([128, NS, 2], BF16, tag="nrm")
                nc.scalar.sqrt(nrmd[:], ss[:].broadcast_to((128, NS, 2)))
                rinvd = stats.tile([128, NS, 2], BF16, tag="rinv")
                nc.vector.reciprocal(rinvd[:], nrmd[:])
                u = upool.tile([128, NS, 128], BF16, tag="u", name=f"u_{g}_{j}")
                nc.vector.tensor_tensor(
                    u[:],
                    pT[:],
                    rinvd[:, :, None].broadcast_to((128, NS, 64, 2)),
                    op=ALU.mult,
                )
                us[g][j], nrms[g][j] = u, nrmd

            def box_stage(g, jj):
                outt = outts[g]
                if jj == 0:
                    srcs = [(wp, 7), (wd, 0), (wd, 1)]
                elif jj == NJ - 1:
                    srcs = [(wd, jj - 1), (wd, jj), (wn, 0)]
                else:
                    srcs = [(wd, jj - 1), (wd, jj), (wd, jj + 1)]
                if True:
                    box = psB.tile([128, NS, 128], BDT, tag="box")
                    for i, (w, sj) in enumerate(srcs):
                        for h in range(2):
                            nc.tensor.matmul(
                                box[:, 4 * h : 4 * h + 4, :],
                                w,
                                us[g][sj][:, 4 * h : 4 * h + 4, :],
                                start=(i == 0),
                                stop=(i == len(srcs) - 1),
                                is_transpose=BOX_IT,
                            )
                    if BOX_IT:
                        boxs = box
                    else:
                        boxs = bxpool.tile([128, NS, 128], BF16, tag="bx")
                        nc.scalar.activation(boxs[:], box[:], AF.Copy)
                    wscr = wpool.tile([128, NS, 128], BF16, tag="w")
                    nc.vector.tensor_tensor(
                        wscr[:], us[g][jj][:], boxs[:], op=ALU.mult
                    )
                    wf = wpool.tile([128, NS, 64], BF16, tag="wf")
                    nc.vector.tensor_tensor(
                        wf[:], wscr[:, :, 0:64], wscr[:, :, 64:128], op=ALU.add
                    )
                    z = stats.tile([128, NS], BF16, tag="z")
                    nc.vector.tensor_reduce(z[:], wf[:], axis=AX.X, op=ALU.add)
                    fsd = stats.tile([128, NS, 2], BF16, tag="fs")
                    nc.vector.tensor_tensor(
                        fsd[:],
                        z[:].broadcast_to((128, NS, 2)),
                        nrms[g][jj][:],
                        op=ALU.mult,
                    )
                    # pair-packed broadcast (fsd lanes equal) so 2x_1p can
                    # trigger; split by e to stay within 3 free dims per AP.
                    for e in range(2):
                        nc.vector.tensor_tensor(
                            outt[:, :, e, jj, :].rearrange(
                                "p n (fo t) -> p n fo t", t=2
                            ),
                            us[g][jj][:, :, e * 64 : (e + 1) * 64].rearrange(
                                "p n (fo t) -> p n fo t", t=2
                            ),
                            fsd[:, :, None].broadcast_to((128, NS, 32, 2)),
                            op=ALU.mult,
                        )

            # self-interleaved schedule: box(g, j-1) rides along norm(g, j),
            # with only the two wrap stages (7, 0) trailing each group; the
            # next group's norm stages queue right behind so engines never
            # drain at group boundaries.
            for g in range(NG):
                norm_stage(g, 0)
                norm_stage(g, 1)
                norm_stage(g, 2)
                for j in range(3, NJ):
                    norm_stage(g, j)
                    box_stage(g, j - 2)
                box_stage(g, NJ - 2)
                box_stage(g, NJ - 1)
                box_stage(g, 0)
                nc.sync.dma_start(out_d[g], outts[g][:])

    nc.compile()
    return nc


def _run(p_vector: np.ndarray, **spmd_kwargs):
    p = np.ascontiguousarray(p_vector, dtype=np.float32)
    assert p.shape == (256, 128, 32, 32)
    # [core, NG, NS, C, S] -> [core, NG, C, NS, S], cast bf16
    shards = (
        p.reshape(8, NG, NS, C, S).transpose(0, 1, 3, 2, 4).astype(NPBF16)
    )
    shards = np.ascontiguousarray(shards)
    perm, wmat = _consts()
    nc = build_nc()
    in_maps = [
        {"p": shards[i], "perm": perm, "wmat": wmat} for i in range(8)
    ]
    return run_bass_kernel_spmd(nc, in_maps, core_ids=list(range(8)), **spmd_kwargs)


def _assemble(res) -> np.ndarray:
    outs = []
    for r in res.results:
        o = np.asarray(r["out"]).astype(np.float32)  # [NG, Q, NS, 2, NJ, 64]
        # -> [NG, NS, 2, Q, NJ, 64] -> [B_local, 2, 1024, 64] -> [B_local, 2048, 64]
        o = o.transpose(0, 2, 3, 1, 4, 5).reshape(B_PER_CORE, 2, 1024, 64)
        outs.append(o.reshape(B_PER_CORE, 2048, 64))
    return np.concatenate(outs, axis=0)


def kernel(p_vector: np.ndarray) -> np.ndarray:
    return _assemble(_run(p_vector))


if __name__ == "__main__":
    x = np.random.randn(256, 128, 32, 32).astype(np.float32)
    y = kernel(x)
    print(y.shape, y.dtype)


# revision 18
# speedup vs baseline: 1.3020x; 1.2854x over previous
"""Trainium2 Bass kernel for nn_BCIM_45861660787130 (pooling / box-filter sim).

Math per sample (C=128 channels, 32x32 spatial = S=1024 pixels):
  unit = p / ||p||_C
  wmean = 3x3 zero-padded box mean of unit (per channel)
  sim = <unit, wmean>_C          # per pixel
  out = p * sim, then channel deinterleave c=(f*2+e) -> [e*S + s, f]

v2 design (bf16 end-to-end, j-interleaved spatial chunking):
  - Host casts p to bf16 and pre-arranges [NG, 128c, NS, 1024s] per core, so
    the input DMA is one fully contiguous 2MB transfer per group.
  - Spatial index is chunked s = 8q + j (q in [0,128) partitions, j in [0,8)
    chunks).  Partition q then holds 8 consecutive output rows, so the output
    DMA per group is one contiguous 16KB-per-partition transfer.
  - PE transpose per (b, j): regular bf16 matmul  out[q, c'] = p_bj^T @ P
    where P is the channel-deinterleave permutation (c=2f+e -> c'=e*64+f),
    folding the output deinterleave into the transpose for free.
  - 3x3 box filter on PE: with j-interleaving the band matrices collapse to
    ONE matrix Wd (offsets {0,+-4} over q) for all chunk pairs except the two
    w-border wraps (j=0 <- chunk 7 via Wp, j=7 <- chunk 0 via Wn).
  - ACT: Square (ss path), sqrt, box PSUM->SBUF bf16 evac.
    DVE: ss-reduce, reciprocal, u-copy (x rinv broadcast), wscr=u*box,
    z-reduce, fs=z*nrm, out=u*fs (broadcast).  All batched over NS=8 samples.
  - out[b] = u * (z*nrm) = p * sim.
"""

import sys

sys.path.insert(0, "/opt/trn_rl_repo")

import numpy as np
import ml_dtypes

from concourse import bacc, mybir, tile
from concourse.bass_utils import run_bass_kernel_spmd

F32 = mybir.dt.float32
BF16 = mybir.dt.bfloat16
AF = mybir.ActivationFunctionType
ALU = mybir.AluOpType
AX = mybir.AxisListType

NPBF16 = ml_dtypes.bfloat16

import os

TRANS_IT = os.environ.get("TRANS_IT", "1") == "1"
BOX_IT = os.environ.get("BOX_IT", "0") == "1"

B_PER_CORE = 32
NS = 8          # samples per group
NG = B_PER_CORE // NS
NJ = 8          # j-chunks (s = 8q + j)
C = 128
S = 1024
Q = 128


def _consts():
    # channel deinterleave permutation: c = 2f+e  ->  c' = e*64 + f
    perm = np.zeros((128, 128), np.float32)
    for e in range(2):
        for f in range(64):
            perm[2 * f + e, e * 64 + f] = 1.0
    # band matrices over q (s = 8q + j; q = h*4 + w//8, so h-step = +-4 in q)
    qq = np.arange(Q)
    wd = np.zeros((Q, Q), np.float32)
    for d in (-4, 0, 4):
        idx = qq + d
        m = (idx >= 0) & (idx < Q)
        wd[idx[m], qq[m]] = 1.0
    wp = np.zeros((Q, Q), np.float32)   # output chunk 0 <- input chunk 7
    for dh in (-1, 0, 1):
        idx = qq - 1 + 4 * dh
        m = (idx >= 0) & (idx < Q) & (qq % 4 >= 1)
        wp[idx[m], qq[m]] = 1.0
    wn = np.zeros((Q, Q), np.float32)   # output chunk 7 <- input chunk 0
    for dh in (-1, 0, 1):
        idx = qq + 1 + 4 * dh
        m = (idx >= 0) & (idx < Q) & (qq % 4 <= 2)
        wn[idx[m], qq[m]] = 1.0
    wmat = (np.stack([wd, wp, wn]) / 9.0).astype(NPBF16)
    return perm.astype(NPBF16), wmat


def build_nc():
    TDT = BF16 if TRANS_IT else F32
    BDT = BF16 if BOX_IT else F32
    nc = bacc.Bacc()
    p_d = nc.declare_dram_parameter("p", [NG, C, NS, S], BF16, isOutput=False)
    out_d = nc.declare_dram_parameter(
        "out", [NG, Q, NS, 2, NJ, 64], BF16, isOutput=True
    )
    perm_d = nc.declare_dram_parameter("perm", [128, 128], BF16, isOutput=False)
    wmat_d = nc.declare_dram_parameter("wmat", [3, 128, 128], BF16, isOutput=False)

    with (
        nc.allow_low_precision(reason="bf16 pipeline; DVE accumulates in f32"),
        tile.TileContext(nc) as tc,
    ):
        with (
            tc.tile_pool(name="consts", bufs=1) as cpool,
            tc.tile_pool(name="pin", bufs=2) as pin,
            tc.tile_pool(name="sq", bufs=3) as sqpool,
            tc.tile_pool(name="upool", bufs=2 * NJ) as upool,
            tc.tile_pool(name="boxs", bufs=3) as bxpool,
            tc.tile_pool(name="wscr", bufs=3) as wpool,
            tc.tile_pool(name="outp", bufs=2) as outpool,
            tc.tile_pool(name="stats", bufs=8 * NJ) as stats,
            tc.tile_pool(name="psT", bufs=2, space="PSUM") as psT,
            tc.tile_pool(name="psB", bufs=3, space="PSUM") as psB,
        ):
            perm = cpool.tile([128, 128], BF16, tag="perm")
            wmat = cpool.tile([128, 3, 128], BF16, tag="wmat")
            nc.sync.dma_start(perm[:], perm_d[:])
            nc.sync.dma_start(wmat[:], wmat_d[:].transpose([1, 0, 2]))
            wd, wp, wn = wmat[:, 0, :], wmat[:, 1, :], wmat[:, 2, :]

            # startup observers: make PE's vector clock see both const-DMA
            # queue sems so steady-state matmuls never wait on them (matmuls
            # only support a single sync wait in codegen).
            scr1 = psT.tile([128, 1], F32, tag="pT")
            nc.tensor.matmul(scr1[:], perm[:], perm[:, 0:1], start=True, stop=True)
            scr2 = psT.tile([128, 1], F32, tag="pT")
            nc.tensor.matmul(scr2[:], perm[:], wmat[:, 0, 0:1], start=True, stop=True)

            pts = []
            for g in range(NG):
                pt = pin.tile([C, NS, S], BF16, tag="pt", name=f"pt_{g}")
                nc.sync.dma_start(pt[:, 0:4, :], p_d[g, :, 0:4, :])
                nc.sync.dma_start(pt[:, 4:8, :], p_d[g, :, 4:8, :])
                pts.append(pt)

            # Per-group state; norm phase of group g+1 is interleaved with
            # the box phase of group g so PE/ACT/DVE never drain between
            # groups.
            outts = [
                outpool.tile([Q, NS, 2, NJ, 64], BF16, tag="ot", name=f"ot_{g}")
                for g in range(NG)
            ]
            us = [dict() for _ in range(NG)]
            nrms = [dict() for _ in range(NG)]

            def norm_stage(g, j):
                pt = pts[g]
                pT = psT.tile([128, NS, 128], TDT, tag="pT")
                for b in range(NS):
                    sl = pt[:, b, :].rearrange("c (q j) -> c j q", j=NJ)[:, j, :]
                    nc.tensor.matmul(
                        pT[:, b, :], sl, perm[:], start=True, stop=True,
                        is_transpose=TRANS_IT,
                    )
                sq = sqpool.tile([128, NS, 128], BF16, tag="sq")
                nc.scalar.activation(sq[:], pT[:], AF.Square)
                sqf = sqpool.tile([128, NS, 64], BF16, tag="sqf")
                nc.vector.tensor_tensor(
                    sqf[:], sq[:, :, 0:64], sq[:, :, 64:128], op=ALU.add
                )
                ss = stats.tile([128, NS], BF16, tag="ss")
                nc.vector.tensor_reduce(ss[:], sqf[:], axis=AX.X, op=ALU.add)
                # duplicated-pair stats: [128, NS, 2] with both lanes equal, so
                # broadcast APs can expose a packed inner [stride 1, 2] dim
                # (2x_1p needs inner step +-1 on every non-scalar operand).
                nrmd = stats.tile([128, NS, 2], BF16, tag="nrm")
                nc.scalar.sqrt(nrmd[:], ss[:].broadcast_to((128, NS, 2)))
                rinvd = stats.tile([128, NS, 2], BF16, tag="rinv")
                nc.vector.reciprocal(rinvd[:], nrmd[:])
                u = upool.tile([128, NS, 128], BF16, tag="u", name=f"u_{g}_{j}")
                nc.vector.tensor_tensor(
                    u[:],
                    pT[:],
                    rinvd[:, :, None].broadcast_to((128, NS, 64, 2)),
                    op=ALU.mult,
                )
                us[g][j], nrms[g][j] = u, nrmd

            def box_stage(g, jj):
                outt = outts[g]
                if jj == 0:
                    srcs = [(wp, 7), (wd, 0), (wd, 1)]
                elif jj == NJ - 1:
                    srcs = [(wd, jj - 1), (wd, jj), (wn, 0)]
                else:
                    srcs = [(wd, jj - 1), (wd, jj), (wd, jj + 1)]
                if True:
                    box = psB.tile([128, NS, 128], BDT, tag="box")
                    for i, (w, sj) in enumerate(srcs):
                        for h in range(2):
                            nc.tensor.matmul(
                                box[:, 4 * h : 4 * h + 4, :],
                                w,
                                us[g][sj][:, 4 * h : 4 * h + 4, :],
                                start=(i == 0),
                                stop=(i == len(srcs) - 1),
                                is_transpose=BOX_IT,
                            )
                    if BOX_IT:
                        boxs = box
                    else:
                        boxs = bxpool.tile([128, NS, 128], BF16, tag="bx")
                        nc.scalar.activation(boxs[:], box[:], AF.Copy)
                    wscr = wpool.tile([128, NS, 128], BF16, tag="w")
                    nc.vector.tensor_tensor(
                        wscr[:], us[g][jj][:], boxs[:], op=ALU.mult
                    )
                    wf = wpool.tile([128, NS, 64], BF16, tag="wf")
                    nc.vector.tensor_tensor(
                        wf[:], wscr[:, :, 0:64], wscr[:, :, 64:128], op=ALU.add
                    )
                    z = stats.tile([128, NS], BF16, tag="z")
                    nc.vector.tensor_reduce(z[:], wf[:], axis=AX.X, op=ALU.add)
                    fsd = stats.tile([128, NS, 2], BF16, tag="fs")
                    nc.vector.tensor_tensor(
                        fsd[:],
                        z[:].broadcast_to((128, NS, 2)),
                        nrms[g][jj][:],
                        op=ALU.mult,
                    )
                    # pair-packed broadcast (fsd lanes equal) so 2x_1p can
                    # trigger; split by e to stay within 3 free dims per AP.
                    for e in range(2):
                        nc.vector.tensor_tensor(
                            outt[:, :, e, jj, :].rearrange(
                                "p n (fo t) -> p n fo t", t=2
                            ),
                            us[g][jj][:, :, e * 64 : (e + 1) * 64].rearrange(
                                "p n (fo t) -> p n fo t", t=2
                            ),
                            fsd[:, :, None].broadcast_to((128, NS, 32, 2)),
                            op=ALU.mult,
                        )

            # self-interleaved schedule: box(g, j-1) rides along norm(g, j),
            # with only the two wrap stages (7, 0) trailing each group; the
            # next group's norm stages queue right behind so engines never
            # drain at group boundaries.
            # cross-group interleave: the box phase of group g is emitted
            # alongside the norm phase of group g+1, so every dependency is a
            # full phase old (strict-FIFO engine queues punish tight chains).
            BOXORD = list(range(1, NJ)) + [0]
            for j in range(NJ):
                norm_stage(0, j)
            for g in range(NG):
                for step in range(NJ):
                    box_stage(g, BOXORD[step])
                    if g + 1 < NG:
                        norm_stage(g + 1, step)
                nc.sync.dma_start(out_d[g], outts[g][:])

    nc.compile()
    return nc


def _run(p_vector: np.ndarray, **spmd_kwargs):
    p = np.ascontiguousarray(p_vector, dtype=np.float32)
    assert p.shape == (256, 128, 32, 32)
    # [core, NG, NS, C, S] -> [core, NG, C, NS, S], cast bf16
    shards = (
        p.reshape(8, NG, NS, C, S).transpose(0, 1, 3, 2, 4).astype(NPBF16)
    )
    shards = np.ascontiguousarray(shards)
    perm, wmat = _consts()
    nc = build_nc()
    in_maps = [
        {"p": shards[i], "perm": perm, "wmat": wmat} for i in range(8)
    ]
    return run_bass_kernel_spmd(nc, in_maps, core_ids=list(range(8)), **spmd_kwargs)


def _assemble(res) -> np.ndarray:
    outs = []
    for r in res.results:
        o = np.asarray(r["out"]).astype(np.float32)  # [NG, Q, NS, 2, NJ, 64]
        # -> [NG, NS, 2, Q, NJ, 64] -> [B_local, 2, 1024, 64] -> [B_local, 2048, 64]
        o = o.transpose(0, 2, 3, 1, 4, 5).reshape(B_PER_CORE, 2, 1024, 64)
        outs.append(o.reshape(B_PER_CORE, 2048, 64))
    return np.concatenate(outs, axis=0)


def kernel(p_vector: np.ndarray) -> np.ndarray:
    return _assemble(_run(p_vector))


if __name__ == "__main__":
    x = np.random.randn(256, 128, 32, 32).astype(np.float32)
    y = kernel(x)
    print(y.shape, y.dtype)
